# revision 8
# baseline (speedup 1.0000x reference)
"""Trainium2 Bass kernel for nn_CSRSparsity (top-k masking autoencoder step).

Math (per reference):
  latents = (x - pre_bias) @ W.T + latent_bias                  [B, H]
  lat_k   = relu(keep top-8 per row of latents)                  (threshold mask)
  tmp[h]  = #{b : h in top8(b) and latents[b,h] > 1e-5}          (all-reduced over cores)
  stats1  = 1 if tmp>0 else stats+1 ; dead1 = stats1 > 30
  x_m     = latents * dead1 (column mask)
  lat_4k  = relu(top-32 of x_m); stats2/dead2 likewise; lat_aux = relu(top-512 of x_m)
  latents_pre_out = x_m * dead2
  recons_* = lat_* @ W + pre_bias ; stats_out = stats2

Top-k is computed as an exact per-row threshold (8th/32nd/512th largest via the
DVE max8 / match_replace instructions); x * (x >= max(t_k, 0)) is identical to
relu of the scattered top-k values (ties at the threshold have measure zero for
continuous data; ties at exactly 0 are absorbed by the relu).

The dead-mask update is a batch-global OR per column: each core reduces its
column "fired" counts, all-reduces them (AllReduce add over 8 cores), then
updates stats identically on every core.  If the dead mask is entirely zero
(x_m == 0 everywhere; the device detects this and branches at runtime, the
condition is identical on all cores since it derives from all-reduced data),
the k=32 / k=512 stages are algebraically zero and are skipped:
lat_4k = lat_aux = latents_pre_out = 0, recons_4k = recons_aux = pre_bias,
stats2 = stats1 + 1.  Otherwise a generic fallback computes them in full.

Data-parallel over batch: 8 cores x 1024 rows.  W, biases, stats replicated.
"""

import numpy as np

import concourse.bass as bass
import concourse.mybir as mybir
import concourse.tile as tile
from concourse import bacc
from concourse.bass_utils import run_bass_kernel_spmd
from concourse.masks import make_identity

F32 = mybir.dt.float32
I32 = mybir.dt.int32
BF16 = mybir.dt.bfloat16
ALU = mybir.AluOpType
AX = mybir.AxisListType

B, D, H = 8192, 1024, 4096
NCORES = 8
BS = B // NCORES          # rows per core = 1024
NB = BS // 128            # 128-row tiles per core = 8
NH = H // 128             # 32 h-chunks of 128
NSL = H // 512            # 8 h-slices of 512
NK = D // 128             # 8 contraction chunks of 128
DEAD_THRESHOLD = 30.0
NEG_BIG = -3.0e38


def _rows(b):
    return slice(b * 128, (b + 1) * 128)


def _emit_topk_select(nc, pools, src_tile, rounds, out_lat_dram, b,
                      fired_acc, ident):
    """From src_tile [128, H] (f32 SBUF), find the per-row (8*rounds)-th
    largest value, build lat = x * (x >= max(t, 0)), DMA it out, transpose it
    to latT [128, NH, 128], and (if fired_acc given) accumulate fired counts.
    Returns the latT tile (consumed by the decode matmuls)."""
    small = pools["small"]
    t8 = small.tile([128, 8], F32, tag="t8")
    if rounds == 1:
        nc.vector.max(out=t8[:], in_=src_tile[:])
    else:
        mr = pools["lat_in"].tile([128, H], F32, tag="lat_in")
        nc.vector.max(out=t8[:], in_=src_tile[:])
        nc.vector.match_replace(out=mr[:], in_to_replace=t8[:],
                                in_values=src_tile[:], imm_value=NEG_BIG)
        for r in range(1, rounds):
            nc.vector.max(out=t8[:], in_=mr[:])
            if r != rounds - 1:
                nc.vector.match_replace(out=mr[:], in_to_replace=t8[:],
                                        in_values=mr[:], imm_value=NEG_BIG)
    thr = small.tile([128, 1], F32, tag="thr")
    nc.vector.tensor_scalar_max(thr[:], t8[:, 7:8], 0.0)

    latk = pools["latk"].tile([128, H], F32, tag="latk")
    # latk = (src >= thr) * src   -- one fused DVE pass
    nc.vector.scalar_tensor_tensor(
        out=latk[:], in0=src_tile[:], scalar=thr[:, 0:1], in1=src_tile[:],
        op0=ALU.is_ge, op1=ALU.mult)
    nc.sync.dma_start(out_lat_dram[_rows(b), :], latk[:])

    latT = pools["latT"].tile([128, NH, 128], F32, tag="latT")
    for c in range(NH):
        pt = pools["ps_tr"].tile([128, 128], F32, tag="ptr")
        nc.tensor.matmul(pt[:], latk[:, c * 128:(c + 1) * 128], ident[:],
                         start=True, stop=True)
        nc.scalar.copy(latT[:, c, :], pt[:])

    if fired_acc is not None:
        # fired condition == any_b(lat > 1e-5).  lat >= 0, so the column max
        # over the batch carries the same information as the indicator sum.
        red = small.tile([128, NH], F32, tag="red")
        nc.vector.tensor_reduce(red[:], latT[:], axis=AX.X, op=ALU.max)
        nc.vector.tensor_max(fired_acc[:], fired_acc[:], red[:])
    return latT


def _emit_decode(nc, pools, latT, wn, pb_bc, out_dram, b):
    """recons[b] = lat @ W + pre_bias (contract h via transposed lat tiles)."""
    ps0 = pools["ps_dec"].tile([128, 512], F32, tag="dec")
    ps1 = pools["ps_dec"].tile([128, 512], F32, tag="dec")
    for c in range(NH):
        nc.tensor.matmul(ps0[:], latT[:, c, :], wn[:, c, 0:512],
                         start=(c == 0), stop=(c == NH - 1))
        nc.tensor.matmul(ps1[:], latT[:, c, :], wn[:, c, 512:1024],
                         start=(c == 0), stop=(c == NH - 1))
    # + pre_bias, evacuated into a free lat_in slot, then DMA out
    rk = pools["lat_in"].tile([128, D], F32, tag="lat_in")
    nc.vector.tensor_add(rk[:, 0:512], ps0[:], pb_bc[:, 0:512])
    nc.vector.tensor_add(rk[:, 512:1024], ps1[:], pb_bc[:, 512:1024])
    nc.sync.dma_start(out_dram[_rows(b), :], rk[:])


def build():
    nc = bacc.Bacc("TRN2", target_bir_lowering=False, debug=False,
                   num_devices=NCORES)

    x_in = nc.dram_tensor("x", [BS, D], F32, kind="ExternalInput")
    w_in = nc.dram_tensor("W", [H, D], F32, kind="ExternalInput")
    pb_in = nc.dram_tensor("pre_bias", [D], F32, kind="ExternalInput")
    lb_in = nc.dram_tensor("latent_bias", [H], F32, kind="ExternalInput")
    st_in = nc.dram_tensor("stats0", [H], I32, kind="ExternalInput")

    o_latk = nc.dram_tensor("o_latk", [BS, H], F32, kind="ExternalOutput")
    o_lat4k = nc.dram_tensor("o_lat4k", [BS, H], F32, kind="ExternalOutput")
    o_lataux = nc.dram_tensor("o_lataux", [BS, H], F32, kind="ExternalOutput")
    o_latpre = nc.dram_tensor("o_latpre", [BS, H], F32, kind="ExternalOutput")
    o_rk = nc.dram_tensor("o_rk", [BS, D], F32, kind="ExternalOutput")
    o_r4k = nc.dram_tensor("o_r4k", [BS, D], F32, kind="ExternalOutput")
    o_raux = nc.dram_tensor("o_raux", [BS, D], F32, kind="ExternalOutput")
    o_stats = nc.dram_tensor("o_stats", [H], I32, kind="ExternalOutput")

    groups = [list(range(NCORES))]

    with tile.TileContext(nc) as tc:
        with (
            tc.tile_pool(name="dram", bufs=1, space="DRAM") as dram,
            tc.tile_pool(name="const", bufs=1) as constp,
            tc.tile_pool(name="small", bufs=1) as small,
            tc.tile_pool(name="ps_tr", bufs=2, space="PSUM") as ps_tr,
            tc.tile_pool(name="ps_misc", bufs=1, space="PSUM") as ps_misc,
        ):
            s_lat = dram.tile([BS, H], F32)
            b1_in = dram.tile([NH, 128], F32)
            b1_out = dram.tile([NH, 128], F32)
            b2_in = dram.tile([NH, 128], F32)
            b2_out = dram.tile([NH, 128], F32)
            dead_row_d = dram.tile([1, H], F32)

            ident = constp.tile([128, 128], F32)
            make_identity(nc, ident[:])
            ones1 = constp.tile([1, 128], F32)
            nc.vector.memset(ones1[:], 1.0)
            ones32 = constp.tile([NH, 1], F32)
            nc.vector.memset(ones32[:], 1.0)
            pb_bc = constp.tile([128, D], F32)
            nc.sync.dma_start(pb_bc[:], pb_in[None, :].to_broadcast([128, D]))
            fired_acc = constp.tile([128, NH], F32)
            nc.vector.memset(fired_acc[:], 0.0)
            fired2 = constp.tile([128, NH], F32)
            nc.vector.memset(fired2[:], 0.0)

            # ---------------- Phase A: encoder ----------------
            with (
                tc.tile_pool(name="xT", bufs=1) as xTp,
                tc.tile_pool(name="stage", bufs=2) as stage,
                tc.tile_pool(name="wt", bufs=2) as wtp,
                tc.tile_pool(name="latstage", bufs=3) as latstage,
                tc.tile_pool(name="abias", bufs=1) as abias,
                tc.tile_pool(name="ps_enc", bufs=3, space="PSUM") as ps_enc,
            ):
                lb_sb = abias.tile([1, H], F32)
                nc.sync.dma_start(lb_sb[:], lb_in[None, :])
                cbias = abias.tile([1, H], F32)
                pbch = abias.tile([128, NK], F32)
                nc.sync.dma_start(
                    pbch[:], pb_in[:].rearrange("(k p) -> p k", p=128))

                # x^T: [d_local, b, k, b_local]
                xT = xTp.tile([128, NB, NK, 128], F32)
                for b in range(NB):
                    xs = stage.tile([128, D], F32, tag="xs")
                    nc.sync.dma_start(xs[:], x_in[_rows(b), :])
                    for k in range(NK):
                        pt = ps_tr.tile([128, 128], F32, tag="ptr")
                        nc.tensor.matmul(
                            pt[:], xs[:, k * 128:(k + 1) * 128], ident[:],
                            start=True, stop=True)
                        nc.scalar.copy(xT[:, b, k, :], pt[:])

                for n in range(NSL):
                    hsl = slice(n * 512, (n + 1) * 512)
                    wt = wtp.tile([128, NK, 512], F32, tag="wt")
                    for wti in range(4):
                        ws = stage.tile([128, D], F32, tag="ws")
                        nc.sync.dma_start(
                            ws[:], w_in[n * 512 + wti * 128:
                                        n * 512 + (wti + 1) * 128, :])
                        for k in range(NK):
                            pt = ps_tr.tile([128, 128], F32, tag="ptr")
                            nc.tensor.matmul(
                                pt[:], ws[:, k * 128:(k + 1) * 128], ident[:],
                                start=True, stop=True)
                            nc.scalar.copy(
                                wt[:, k, wti * 128:(wti + 1) * 128], pt[:])
                    # cbias slice = latent_bias - pre_bias @ W^T
                    cps = ps_misc.tile([1, 512], F32, tag="mi")
                    for k in range(NK):
                        nc.tensor.matmul(cps[:], pbch[:, k:k + 1], wt[:, k, :],
                                         start=(k == 0), stop=(k == NK - 1))
                    nc.vector.scalar_tensor_tensor(
                        out=cbias[:, hsl], in0=cps[:], scalar=-1.0,
                        in1=lb_sb[:, hsl], op0=ALU.mult, op1=ALU.add)
                    for b in range(NB):
                        ps = ps_enc.tile([128, 512], F32, tag="enc")
                        nc.tensor.matmul(ps[:], ones1[:], cbias[0:1, hsl],
                                         start=True, stop=False)
                        for k in range(NK):
                            nc.tensor.matmul(ps[:], xT[:, b, k, :],
                                             wt[:, k, :],
                                             start=False, stop=(k == NK - 1))
                        lt = latstage.tile([128, 512], F32, tag="lt")
                        nc.scalar.copy(lt[:], ps[:])
                        nc.sync.dma_start(s_lat[_rows(b), hsl], lt[:])

            # ---------------- Phase B: select + decode ----------------
            with (
                tc.tile_pool(name="wn", bufs=1) as wnp,
                tc.tile_pool(name="lat_in", bufs=2) as lat_in_p,
                tc.tile_pool(name="latk", bufs=1) as latkp,
                tc.tile_pool(name="latT", bufs=1) as latTp,
                tc.tile_pool(name="ps_dec", bufs=2, space="PSUM") as ps_dec,
            ):
                pools = {"small": small, "lat_in": lat_in_p, "latk": latkp,
                         "latT": latTp, "ps_tr": ps_tr, "ps_dec": ps_dec}

                wn = wnp.tile([128, NH, D], F32)
                nc.sync.dma_start(
                    wn[:], w_in[:].rearrange("(c p) d -> p c d", p=128))

                for b in range(NB):
                    lat = lat_in_p.tile([128, H], F32, tag="lat_in")
                    nc.sync.dma_start(lat[:], s_lat[_rows(b), :])
                    latT = _emit_topk_select(
                        nc, pools, lat, 1, o_latk, b, fired_acc, ident)
                    _emit_decode(nc, pools, latT, wn, pb_bc, o_rk, b)

                # fired -> [NH,128] -> DRAM -> AllReduce -> back
                pf = ps_misc.tile([NH, 128], F32, tag="mi")
                nc.tensor.matmul(pf[:], fired_acc[:], ident[:],
                                 start=True, stop=True)
                firedT = small.tile([NH, 128], F32, tag="firedT")
                nc.scalar.copy(firedT[:], pf[:])
                nc.sync.dma_start(b1_in[:], firedT[:])
                nc.gpsimd.collective_compute(
                    "AllReduce", ALU.max, replica_groups=groups,
                    ins=[b1_in.opt()], outs=[b1_out.opt()])
                ftot = small.tile([NH, 128], F32, tag="ftot")
                nc.sync.dma_start(ftot[:], b1_out[:])

                # stats1 = stats*(1 - min(tmp,1)) + 1 ; dead1 = stats1 > 30
                st_raw = small.tile([NH, 128], I32, tag="st_raw")
                nc.sync.dma_start(
                    st_raw[:], st_in[:].rearrange("(a b) -> a b", a=NH))
                st_f = small.tile([NH, 128], F32, tag="st_f")
                nc.vector.tensor_copy(st_f[:], st_raw[:])
                om = small.tile([NH, 128], F32, tag="om")
                nc.vector.tensor_scalar(om[:], ftot[:], 1e-5, None,
                                        op0=ALU.is_gt)
                nc.vector.tensor_scalar(om[:], om[:], -1.0, 1.0,
                                        op0=ALU.mult, op1=ALU.add)
                stats1 = small.tile([NH, 128], F32, tag="stats1")
                nc.vector.tensor_mul(stats1[:], st_f[:], om[:])
                nc.vector.tensor_scalar_add(stats1[:], stats1[:], 1.0)
                dead1 = small.tile([NH, 128], F32, tag="dead1")
                nc.vector.tensor_scalar(dead1[:], stats1[:], DEAD_THRESHOLD,
                                        None, op0=ALU.is_gt)
                nd = small.tile([NH, 1], F32, tag="nd")
                nc.vector.tensor_reduce(nd[:], dead1[:], axis=AX.X,
                                        op=ALU.add)
                fps = ps_misc.tile([1, 1], F32, tag="mi")
                nc.tensor.matmul(fps[:], nd[:], ones32[:],
                                 start=True, stop=True)
                flag_f = small.tile([1, 1], F32, tag="flag_f")
                nc.scalar.copy(flag_f[:], fps[:])
                flag_i = small.tile([1, 1], I32, tag="flag_i")
                nc.vector.tensor_scalar(flag_i[:], flag_f[:], 0.5, None,
                                        op0=ALU.is_gt)

                regs = nc.alloc_registers("deadflag")
                nc.regs_load(regs, flag_i[0:1, 0:1])
                rv = nc.snap(regs, donate=True, min_val=0, max_val=1)

                with tc.If(rv == 0) as cmp:
                    # Fast path: dead1 == 0 everywhere => x_m == 0 =>
                    # lat_4k = lat_aux = latents_pre = 0 (outputs pre-zeroed),
                    # recons_4k = recons_aux = pre_bias, stats2 = stats1 + 1.
                    s2 = small.tile([NH, 128], F32, tag="s2")
                    nc.vector.tensor_scalar_add(s2[:], stats1[:], 1.0)
                    statsout = small.tile([NH, 128], I32, tag="statsout")
                    nc.vector.tensor_copy(statsout[:], s2[:])
                    nc.sync.dma_start(
                        o_stats[:].rearrange("(a b) -> a b", a=NH),
                        statsout[:])
                    for b in range(NB):
                        nc.sync.dma_start(o_r4k[_rows(b), :], pb_bc[:])
                        nc.sync.dma_start(o_raux[_rows(b), :], pb_bc[:])
                with cmp.Else():
                    _emit_generic(nc, pools, small, s_lat, stats1, fired2,
                                  wn, pb_bc, ident, groups,
                                  b2_in, b2_out, dead_row_d, dead1,
                                  o_lat4k, o_lataux, o_latpre, o_r4k,
                                  o_raux, o_stats)

    nc.compile()
    return nc


def _emit_generic(nc, pools, small, s_lat, stats1, fired2, wn, pb_bc, ident,
                  groups, b2_in, b2_out, dead_row_d, dead1,
                  o_lat4k, o_lataux, o_latpre, o_r4k, o_raux, o_stats):
    """Generic fallback: some columns are dead -> full k=32 / k=512 stages."""

    def mask_inplace(dead_t, out_dram=None):
        # dead [NH,128] -> DRAM row -> partition-broadcast [128, H];
        # then for each row tile: x_m(b) = s_lat(b) * mask, stored back
        # (or to out_dram when given).
        nc.sync.dma_start(
            dead_row_d[0, :].rearrange("(a b) -> a b", a=NH), dead_t[:])
        bc = pools["latk"].tile([128, H], F32, tag="latk")
        nc.sync.dma_start(bc[:], dead_row_d[0:1, :].to_broadcast([128, H]))
        for b in range(NB):
            xm = pools["lat_in"].tile([128, H], F32, tag="lat_in")
            nc.sync.dma_start(xm[:], s_lat[_rows(b), :])
            nc.vector.tensor_mul(xm[:], xm[:], bc[:])
            dst = s_lat if out_dram is None else out_dram
            nc.sync.dma_start(dst[_rows(b), :], xm[:])

    mask_inplace(dead1)

    # k=32 select + decode + fired2
    for b in range(NB):
        xm = pools["lat_in"].tile([128, H], F32, tag="lat_in")
        nc.sync.dma_start(xm[:], s_lat[_rows(b), :])
        latT = _emit_topk_select(nc, pools, xm, 4, o_lat4k, b, fired2, ident)
        _emit_decode(nc, pools, latT, wn, pb_bc, o_r4k, b)

    pf = pools["ps_tr"].tile([NH, 128], F32, tag="ptr")
    nc.tensor.matmul(pf[:], fired2[:], ident[:], start=True, stop=True)
    f2T = small.tile([NH, 128], F32, tag="f2T")
    nc.scalar.copy(f2T[:], pf[:])
    nc.sync.dma_start(b2_in[:], f2T[:])
    nc.gpsimd.collective_compute(
        "AllReduce", ALU.max, replica_groups=groups,
        ins=[b2_in.opt()], outs=[b2_out.opt()])
    f2tot = small.tile([NH, 128], F32, tag="f2tot")
    nc.sync.dma_start(f2tot[:], b2_out[:])

    om2 = small.tile([NH, 128], F32, tag="om2")
    nc.vector.tensor_scalar(om2[:], f2tot[:], 1e-5, None,
                            op0=ALU.is_gt)
    nc.vector.tensor_scalar(om2[:], om2[:], -1.0, 1.0,
                            op0=ALU.mult, op1=ALU.add)
    stats2 = small.tile([NH, 128], F32, tag="stats2g")
    nc.vector.tensor_mul(stats2[:], stats1[:], om2[:])
    nc.vector.tensor_scalar_add(stats2[:], stats2[:], 1.0)
    statsout = small.tile([NH, 128], I32, tag="statsoutg")
    nc.vector.tensor_copy(statsout[:], stats2[:])
    nc.sync.dma_start(o_stats[:].rearrange("(a b) -> a b", a=NH), statsout[:])
    dead2 = small.tile([NH, 128], F32, tag="dead2")
    nc.vector.tensor_scalar(dead2[:], stats2[:], DEAD_THRESHOLD, None,
                            op0=ALU.is_gt)

    # latents_pre_out = x_m * dead2 (does NOT modify s_lat)
    mask_inplace(dead2, out_dram=o_latpre)

    # k=512 aux select + decode (no stats update)
    for b in range(NB):
        xm = pools["lat_in"].tile([128, H], F32, tag="lat_in")
        nc.sync.dma_start(xm[:], s_lat[_rows(b), :])
        latT = _emit_topk_select(nc, pools, xm, 64, o_lataux, b, None, ident)
        _emit_decode(nc, pools, latT, wn, pb_bc, o_raux, b)


_CACHED = None


def _get_nc():
    global _CACHED
    if _CACHED is None:
        _CACHED = build()
    return _CACHED


def kernel(x, W, pre_bias, latent_bias, stats_last_nonzero):
    x = np.ascontiguousarray(x, dtype=np.float32)
    W = np.ascontiguousarray(W, dtype=np.float32)
    pre_bias = np.ascontiguousarray(pre_bias, dtype=np.float32)
    latent_bias = np.ascontiguousarray(latent_bias, dtype=np.float32)
    stats = np.ascontiguousarray(stats_last_nonzero, dtype=np.int32)

    nc = _get_nc()
    in_maps = [
        {"x": x[i * BS:(i + 1) * BS], "W": W, "pre_bias": pre_bias,
         "latent_bias": latent_bias, "stats0": stats}
        for i in range(NCORES)
    ]
    res = run_bass_kernel_spmd(nc, in_maps, core_ids=list(range(NCORES)))
    rs = res.results
    cat = lambda name: np.concatenate([rs[i][name] for i in range(NCORES)],
                                      axis=0)
    return (cat("o_latk"), cat("o_lat4k"), cat("o_lataux"), cat("o_latpre"),
            cat("o_rk"), cat("o_r4k"), cat("o_raux"), rs[0]["o_stats"])


# revision 9
# speedup vs baseline: 1.1669x; 1.1669x over previous
"""Trainium2 Bass kernel for nn_CSRSparsity (top-k masking autoencoder step).

Math (per reference):
  latents = (x - pre_bias) @ W.T + latent_bias                  [B, H]
  lat_k   = relu(keep top-8 per row of latents)                  (threshold mask)
  tmp[h]  = #{b : h in top8(b) and latents[b,h] > 1e-5}          (all-reduced over cores)
  stats1  = 1 if tmp>0 else stats+1 ; dead1 = stats1 > 30
  x_m     = latents * dead1 (column mask)
  lat_4k  = relu(top-32 of x_m); stats2/dead2 likewise; lat_aux = relu(top-512 of x_m)
  latents_pre_out = x_m * dead2
  recons_* = lat_* @ W + pre_bias ; stats_out = stats2

Top-k is computed as an exact per-row threshold (8th/32nd/512th largest via the
DVE max8 / match_replace instructions); x * (x >= max(t_k, 0)) is identical to
relu of the scattered top-k values (ties at the threshold have measure zero for
continuous data; ties at exactly 0 are absorbed by the relu).

The dead-mask update is a batch-global OR per column: each core reduces its
column "fired" counts, all-reduces them (AllReduce add over 8 cores), then
updates stats identically on every core.  If the dead mask is entirely zero
(x_m == 0 everywhere; the device detects this and branches at runtime, the
condition is identical on all cores since it derives from all-reduced data),
the k=32 / k=512 stages are algebraically zero and are skipped:
lat_4k = lat_aux = latents_pre_out = 0, recons_4k = recons_aux = pre_bias,
stats2 = stats1 + 1.  Otherwise a generic fallback computes them in full.

Data-parallel over batch: 8 cores x 1024 rows.  W, biases, stats replicated.
"""

import numpy as np

import concourse.bass as bass
import concourse.mybir as mybir
import concourse.tile as tile
from concourse import bacc
from concourse.bass_utils import run_bass_kernel_spmd
from concourse.masks import make_identity

F32 = mybir.dt.float32
I32 = mybir.dt.int32
BF16 = mybir.dt.bfloat16
ALU = mybir.AluOpType
AX = mybir.AxisListType

B, D, H = 8192, 1024, 4096
NCORES = 8
BS = B // NCORES          # rows per core = 1024
NB = BS // 128            # 128-row tiles per core = 8
NH = H // 128             # 32 h-chunks of 128
NSL = H // 512            # 8 h-slices of 512
NK = D // 128             # 8 contraction chunks of 128
DEAD_THRESHOLD = 30.0
NEG_BIG = -3.0e38


def _rows(b):
    return slice(b * 128, (b + 1) * 128)


def _emit_topk_select(nc, pools, src_tile, rounds, out_lat_dram, b,
                      fired_acc, ident):
    """From src_tile [128, H] (f32 SBUF), find the per-row (8*rounds)-th
    largest value, build lat = x * (x >= max(t, 0)), DMA it out, transpose it
    to latT [128, NH, 128], and (if fired_acc given) accumulate fired counts.
    Returns the latT tile (consumed by the decode matmuls)."""
    small = pools["small"]
    t8 = small.tile([128, 8], F32, tag="t8")
    if rounds == 1:
        nc.vector.max(out=t8[:], in_=src_tile[:])
    else:
        mr = pools["lat_in"].tile([128, H], F32, tag="lat_in")
        nc.vector.max(out=t8[:], in_=src_tile[:])
        nc.vector.match_replace(out=mr[:], in_to_replace=t8[:],
                                in_values=src_tile[:], imm_value=NEG_BIG)
        for r in range(1, rounds):
            nc.vector.max(out=t8[:], in_=mr[:])
            if r != rounds - 1:
                nc.vector.match_replace(out=mr[:], in_to_replace=t8[:],
                                        in_values=mr[:], imm_value=NEG_BIG)
    thr = small.tile([128, 1], F32, tag="thr")
    nc.vector.tensor_scalar_max(thr[:], t8[:, 7:8], 0.0)

    latk = pools["latk"].tile([128, H], F32, tag="latk")
    # latk = (src >= thr) * src   -- one fused DVE pass
    nc.vector.scalar_tensor_tensor(
        out=latk[:], in0=src_tile[:], scalar=thr[:, 0:1], in1=src_tile[:],
        op0=ALU.is_ge, op1=ALU.mult)
    nc.sync.dma_start(out_lat_dram[_rows(b), :], latk[:])

    latT = pools["latT"].tile([128, NH, 128], F32, tag="latT")
    for c in range(NH):
        pt = pools["ps_tr"].tile([128, 128], F32, tag="ptr")
        nc.tensor.matmul(pt[:], latk[:, c * 128:(c + 1) * 128], ident[:],
                         start=True, stop=True)
        nc.scalar.copy(latT[:, c, :], pt[:])

    if fired_acc is not None:
        # fired condition == any_b(lat > 1e-5).  lat >= 0, so the column max
        # over the batch carries the same information as the indicator sum.
        red = small.tile([128, NH], F32, tag="red")
        nc.vector.tensor_reduce(red[:], latT[:], axis=AX.X, op=ALU.max)
        nc.vector.tensor_max(fired_acc[:], fired_acc[:], red[:])
    return latT


def _emit_decode(nc, pools, latT, wn, pb_bc, out_dram, b):
    """recons[b] = lat @ W + pre_bias (contract h via transposed lat tiles)."""
    ps0 = pools["ps_dec"].tile([128, 512], F32, tag="dec")
    ps1 = pools["ps_dec"].tile([128, 512], F32, tag="dec")
    for c in range(NH):
        nc.tensor.matmul(ps0[:], latT[:, c, :], wn[:, c, 0:512],
                         start=(c == 0), stop=(c == NH - 1))
        nc.tensor.matmul(ps1[:], latT[:, c, :], wn[:, c, 512:1024],
                         start=(c == 0), stop=(c == NH - 1))
    # + pre_bias, evacuated into a free lat_in slot, then DMA out
    rk = pools["lat_in"].tile([128, D], F32, tag="lat_in")
    nc.vector.tensor_add(rk[:, 0:512], ps0[:], pb_bc[:, 0:512])
    nc.vector.tensor_add(rk[:, 512:1024], ps1[:], pb_bc[:, 512:1024])
    nc.sync.dma_start(out_dram[_rows(b), :], rk[:])


def build():
    nc = bacc.Bacc("TRN2", target_bir_lowering=False, debug=False,
                   num_devices=NCORES)

    x_in = nc.dram_tensor("x", [BS, D], F32, kind="ExternalInput")
    w_in = nc.dram_tensor("W", [H, D], F32, kind="ExternalInput")
    pb_in = nc.dram_tensor("pre_bias", [D], F32, kind="ExternalInput")
    lb_in = nc.dram_tensor("latent_bias", [H], F32, kind="ExternalInput")
    st_in = nc.dram_tensor("stats0", [H], I32, kind="ExternalInput")

    o_latk = nc.dram_tensor("o_latk", [BS, H], F32, kind="ExternalOutput")
    o_lat4k = nc.dram_tensor("o_lat4k", [BS, H], F32, kind="ExternalOutput")
    o_lataux = nc.dram_tensor("o_lataux", [BS, H], F32, kind="ExternalOutput")
    o_latpre = nc.dram_tensor("o_latpre", [BS, H], F32, kind="ExternalOutput")
    o_rk = nc.dram_tensor("o_rk", [BS, D], F32, kind="ExternalOutput")
    o_r4k = nc.dram_tensor("o_r4k", [BS, D], F32, kind="ExternalOutput")
    o_raux = nc.dram_tensor("o_raux", [BS, D], F32, kind="ExternalOutput")
    o_stats = nc.dram_tensor("o_stats", [H], I32, kind="ExternalOutput")

    groups = [list(range(NCORES))]

    with tile.TileContext(nc) as tc:
        with (
            tc.tile_pool(name="dram", bufs=1, space="DRAM") as dram,
            tc.tile_pool(name="const", bufs=1) as constp,
            tc.tile_pool(name="small", bufs=1) as small,
            tc.tile_pool(name="ps_tr", bufs=3, space="PSUM") as ps_tr,
            tc.tile_pool(name="ps_misc", bufs=1, space="PSUM") as ps_misc,
        ):
            s_lat = dram.tile([BS, H], F32)
            b1_in = dram.tile([NH, 128], F32)
            b1_out = dram.tile([NH, 128], F32)
            b2_in = dram.tile([NH, 128], F32)
            b2_out = dram.tile([NH, 128], F32)
            dead_row_d = dram.tile([1, H], F32)

            ident = constp.tile([128, 128], F32)
            make_identity(nc, ident[:])
            ones32 = constp.tile([NH, 1], F32)
            nc.vector.memset(ones32[:], 1.0)
            pb_bc = constp.tile([128, D], F32)
            nc.sync.dma_start(pb_bc[:], pb_in[None, :].to_broadcast([128, D]))
            fired_acc = constp.tile([128, NH], F32)
            nc.vector.memset(fired_acc[:], 0.0)
            fired2 = constp.tile([128, NH], F32)
            nc.vector.memset(fired2[:], 0.0)

            # ---------------- Phase A: encoder ----------------
            with (
                tc.tile_pool(name="xT", bufs=1) as xTp,
                tc.tile_pool(name="stage", bufs=2) as stage,
                tc.tile_pool(name="wt", bufs=2) as wtp,
                tc.tile_pool(name="latstage", bufs=3) as latstage,
                tc.tile_pool(name="abias", bufs=1) as abias,
                tc.tile_pool(name="ps_enc", bufs=4, space="PSUM") as ps_enc,
            ):
                lb_bc = abias.tile([128, H], F32)
                nc.sync.dma_start(lb_bc[:],
                                  lb_in[None, :].to_broadcast([128, H]))

                # x^T: [d_local, b, k, b_local]
                xT = xTp.tile([128, NB, NK, 128], F32)
                for b in range(NB):
                    xs = stage.tile([128, D], F32, tag="xs")
                    nc.sync.dma_start(xs[:], x_in[_rows(b), :])
                    nc.vector.tensor_sub(xs[:], xs[:], pb_bc[:])
                    for k in range(NK):
                        pt = ps_tr.tile([128, 128], F32, tag="ptr")
                        nc.tensor.matmul(
                            pt[:], xs[:, k * 128:(k + 1) * 128], ident[:],
                            start=True, stop=True)
                        nc.scalar.copy(xT[:, b, k, :], pt[:])

                for n in range(NSL):
                    hsl = slice(n * 512, (n + 1) * 512)
                    wt = wtp.tile([128, NK, 512], F32, tag="wt")
                    for wti in range(4):
                        ws = stage.tile([128, D], F32, tag="ws")
                        nc.sync.dma_start(
                            ws[:], w_in[n * 512 + wti * 128:
                                        n * 512 + (wti + 1) * 128, :])
                        for k in range(NK):
                            pt = ps_tr.tile([128, 128], F32, tag="ptr")
                            nc.tensor.matmul(
                                pt[:], ws[:, k * 128:(k + 1) * 128], ident[:],
                                start=True, stop=True)
                            nc.scalar.copy(
                                wt[:, k, wti * 128:(wti + 1) * 128], pt[:])
                    for b in range(NB):
                        ps = ps_enc.tile([128, 512], F32, tag="enc")
                        for k in range(NK):
                            nc.tensor.matmul(ps[:], xT[:, b, k, :],
                                             wt[:, k, :],
                                             start=(k == 0),
                                             stop=(k == NK - 1))
                        lt = latstage.tile([128, 512], F32, tag="lt")
                        nc.vector.tensor_add(lt[:], ps[:], lb_bc[:, hsl])
                        nc.sync.dma_start(s_lat[_rows(b), hsl], lt[:])

            # ---------------- Phase B: select + decode ----------------
            with (
                tc.tile_pool(name="wn", bufs=1) as wnp,
                tc.tile_pool(name="lat_in", bufs=2) as lat_in_p,
                tc.tile_pool(name="latk", bufs=1) as latkp,
                tc.tile_pool(name="latT", bufs=1) as latTp,
                tc.tile_pool(name="ps_dec", bufs=2, space="PSUM") as ps_dec,
            ):
                pools = {"small": small, "lat_in": lat_in_p, "latk": latkp,
                         "latT": latTp, "ps_tr": ps_tr, "ps_dec": ps_dec}

                wn = wnp.tile([128, NH, D], F32)
                nc.sync.dma_start(
                    wn[:], w_in[:].rearrange("(c p) d -> p c d", p=128))

                for b in range(NB):
                    lat = lat_in_p.tile([128, H], F32, tag="lat_in")
                    nc.sync.dma_start(lat[:], s_lat[_rows(b), :])
                    latT = _emit_topk_select(
                        nc, pools, lat, 1, o_latk, b, fired_acc, ident)
                    _emit_decode(nc, pools, latT, wn, pb_bc, o_rk, b)

                # fired -> [NH,128] -> DRAM -> AllReduce -> back
                pf = ps_misc.tile([NH, 128], F32, tag="mi")
                nc.tensor.matmul(pf[:], fired_acc[:], ident[:],
                                 start=True, stop=True)
                firedT = small.tile([NH, 128], F32, tag="firedT")
                nc.scalar.copy(firedT[:], pf[:])
                nc.sync.dma_start(b1_in[:], firedT[:])
                nc.gpsimd.collective_compute(
                    "AllReduce", ALU.max, replica_groups=groups,
                    ins=[b1_in.opt()], outs=[b1_out.opt()])
                ftot = small.tile([NH, 128], F32, tag="ftot")
                nc.sync.dma_start(ftot[:], b1_out[:])

                # stats1 = stats*(1 - min(tmp,1)) + 1 ; dead1 = stats1 > 30
                st_raw = small.tile([NH, 128], I32, tag="st_raw")
                nc.sync.dma_start(
                    st_raw[:], st_in[:].rearrange("(a b) -> a b", a=NH))
                st_f = small.tile([NH, 128], F32, tag="st_f")
                nc.vector.tensor_copy(st_f[:], st_raw[:])
                om = small.tile([NH, 128], F32, tag="om")
                nc.vector.tensor_scalar(om[:], ftot[:], 1e-5, None,
                                        op0=ALU.is_gt)
                nc.vector.tensor_scalar(om[:], om[:], -1.0, 1.0,
                                        op0=ALU.mult, op1=ALU.add)
                stats1 = small.tile([NH, 128], F32, tag="stats1")
                nc.vector.tensor_mul(stats1[:], st_f[:], om[:])
                nc.vector.tensor_scalar_add(stats1[:], stats1[:], 1.0)
                dead1 = small.tile([NH, 128], F32, tag="dead1")
                nc.vector.tensor_scalar(dead1[:], stats1[:], DEAD_THRESHOLD,
                                        None, op0=ALU.is_gt)
                nd = small.tile([NH, 1], F32, tag="nd")
                nc.vector.tensor_reduce(nd[:], dead1[:], axis=AX.X,
                                        op=ALU.add)
                fps = ps_misc.tile([1, 1], F32, tag="mi")
                nc.tensor.matmul(fps[:], nd[:], ones32[:],
                                 start=True, stop=True)
                flag_f = small.tile([1, 1], F32, tag="flag_f")
                nc.scalar.copy(flag_f[:], fps[:])
                flag_i = small.tile([1, 1], I32, tag="flag_i")
                nc.vector.tensor_scalar(flag_i[:], flag_f[:], 0.5, None,
                                        op0=ALU.is_gt)

                regs = nc.alloc_registers("deadflag")
                nc.regs_load(regs, flag_i[0:1, 0:1])
                rv = nc.snap(regs, donate=True, min_val=0, max_val=1)

                with tc.If(rv == 0) as cmp:
                    # Fast path: dead1 == 0 everywhere => x_m == 0 =>
                    # lat_4k = lat_aux = latents_pre = 0 (outputs pre-zeroed),
                    # recons_4k = recons_aux = pre_bias, stats2 = stats1 + 1.
                    s2 = small.tile([NH, 128], F32, tag="s2")
                    nc.vector.tensor_scalar_add(s2[:], stats1[:], 1.0)
                    statsout = small.tile([NH, 128], I32, tag="statsout")
                    nc.vector.tensor_copy(statsout[:], s2[:])
                    nc.sync.dma_start(
                        o_stats[:].rearrange("(a b) -> a b", a=NH),
                        statsout[:])
                    for b in range(NB):
                        nc.sync.dma_start(o_r4k[_rows(b), :], pb_bc[:])
                        nc.sync.dma_start(o_raux[_rows(b), :], pb_bc[:])
                with cmp.Else():
                    _emit_generic(nc, pools, small, s_lat, stats1, fired2,
                                  wn, pb_bc, ident, groups,
                                  b2_in, b2_out, dead_row_d, dead1,
                                  o_lat4k, o_lataux, o_latpre, o_r4k,
                                  o_raux, o_stats)

    nc.compile()
    return nc


def _emit_generic(nc, pools, small, s_lat, stats1, fired2, wn, pb_bc, ident,
                  groups, b2_in, b2_out, dead_row_d, dead1,
                  o_lat4k, o_lataux, o_latpre, o_r4k, o_raux, o_stats):
    """Generic fallback: some columns are dead -> full k=32 / k=512 stages."""

    def mask_inplace(dead_t, out_dram=None):
        # dead [NH,128] -> DRAM row -> partition-broadcast [128, H];
        # then for each row tile: x_m(b) = s_lat(b) * mask, stored back
        # (or to out_dram when given).
        nc.sync.dma_start(
            dead_row_d[0, :].rearrange("(a b) -> a b", a=NH), dead_t[:])
        bc = pools["latk"].tile([128, H], F32, tag="latk")
        nc.sync.dma_start(bc[:], dead_row_d[0:1, :].to_broadcast([128, H]))
        for b in range(NB):
            xm = pools["lat_in"].tile([128, H], F32, tag="lat_in")
            nc.sync.dma_start(xm[:], s_lat[_rows(b), :])
            nc.vector.tensor_mul(xm[:], xm[:], bc[:])
            dst = s_lat if out_dram is None else out_dram
            nc.sync.dma_start(dst[_rows(b), :], xm[:])

    mask_inplace(dead1)

    # k=32 select + decode + fired2
    for b in range(NB):
        xm = pools["lat_in"].tile([128, H], F32, tag="lat_in")
        nc.sync.dma_start(xm[:], s_lat[_rows(b), :])
        latT = _emit_topk_select(nc, pools, xm, 4, o_lat4k, b, fired2, ident)
        _emit_decode(nc, pools, latT, wn, pb_bc, o_r4k, b)

    pf = pools["ps_tr"].tile([NH, 128], F32, tag="ptr")
    nc.tensor.matmul(pf[:], fired2[:], ident[:], start=True, stop=True)
    f2T = small.tile([NH, 128], F32, tag="f2T")
    nc.scalar.copy(f2T[:], pf[:])
    nc.sync.dma_start(b2_in[:], f2T[:])
    nc.gpsimd.collective_compute(
        "AllReduce", ALU.max, replica_groups=groups,
        ins=[b2_in.opt()], outs=[b2_out.opt()])
    f2tot = small.tile([NH, 128], F32, tag="f2tot")
    nc.sync.dma_start(f2tot[:], b2_out[:])

    om2 = small.tile([NH, 128], F32, tag="om2")
    nc.vector.tensor_scalar(om2[:], f2tot[:], 1e-5, None,
                            op0=ALU.is_gt)
    nc.vector.tensor_scalar(om2[:], om2[:], -1.0, 1.0,
                            op0=ALU.mult, op1=ALU.add)
    stats2 = small.tile([NH, 128], F32, tag="stats2g")
    nc.vector.tensor_mul(stats2[:], stats1[:], om2[:])
    nc.vector.tensor_scalar_add(stats2[:], stats2[:], 1.0)
    statsout = small.tile([NH, 128], I32, tag="statsoutg")
    nc.vector.tensor_copy(statsout[:], stats2[:])
    nc.sync.dma_start(o_stats[:].rearrange("(a b) -> a b", a=NH), statsout[:])
    dead2 = small.tile([NH, 128], F32, tag="dead2")
    nc.vector.tensor_scalar(dead2[:], stats2[:], DEAD_THRESHOLD, None,
                            op0=ALU.is_gt)

    # latents_pre_out = x_m * dead2 (does NOT modify s_lat)
    mask_inplace(dead2, out_dram=o_latpre)

    # k=512 aux select + decode (no stats update)
    for b in range(NB):
        xm = pools["lat_in"].tile([128, H], F32, tag="lat_in")
        nc.sync.dma_start(xm[:], s_lat[_rows(b), :])
        latT = _emit_topk_select(nc, pools, xm, 64, o_lataux, b, None, ident)
        _emit_decode(nc, pools, latT, wn, pb_bc, o_raux, b)


_CACHED = None


def _get_nc():
    global _CACHED
    if _CACHED is None:
        _CACHED = build()
    return _CACHED


def kernel(x, W, pre_bias, latent_bias, stats_last_nonzero):
    x = np.ascontiguousarray(x, dtype=np.float32)
    W = np.ascontiguousarray(W, dtype=np.float32)
    pre_bias = np.ascontiguousarray(pre_bias, dtype=np.float32)
    latent_bias = np.ascontiguousarray(latent_bias, dtype=np.float32)
    stats = np.ascontiguousarray(stats_last_nonzero, dtype=np.int32)

    nc = _get_nc()
    in_maps = [
        {"x": x[i * BS:(i + 1) * BS], "W": W, "pre_bias": pre_bias,
         "latent_bias": latent_bias, "stats0": stats}
        for i in range(NCORES)
    ]
    res = run_bass_kernel_spmd(nc, in_maps, core_ids=list(range(NCORES)))
    rs = res.results
    cat = lambda name: np.concatenate([rs[i][name] for i in range(NCORES)],
                                      axis=0)
    return (cat("o_latk"), cat("o_lat4k"), cat("o_lataux"), cat("o_latpre"),
            cat("o_rk"), cat("o_r4k"), cat("o_raux"), rs[0]["o_stats"])
